# revision 18
# baseline (speedup 1.0000x reference)
"""MoE (6 experts, 2 fixed + top-2-of-4 variable) + LayerNorm Trainium2 kernel.

Sharding: each of the 8 cores owns a 512-wide d_ff slice of ALL 6 experts
(W1[:, :, 512c:512(c+1)], W2[:, 512c:512(c+1), :]).  Every core computes its
partial FFN output for all 4096 tokens (masked per-token for the variable
experts), partials are summed with a ReduceScatter, and each core LayerNorms
its own 512-token slice of the result.

Routing (softmax top-2 over 4 variable experts) is computed on host exactly
mirroring the reference, producing a {0,1} mask per (expert, token); the /4
combine divide is folded into W2 and b2 on host (exact power-of-2 scaling).
"""

import os

import numpy as np

NUM_EXPERTS = 6
VAR_EXPERTS = 4
FIXED_EXPERTS = 2
TOP_K = 2
D = 1024
F = 4096
N_CORES = 8
B, S = 2, 2048
NTOK = B * S  # 4096
FC = F // N_CORES  # 512 dff slice per core
TOKC = 256  # token chunk per inner iteration
NTC = NTOK // TOKC  # 16
TOK_SLICE = NTOK // N_CORES  # 512 tokens per core after reduce-scatter
LN_EPS = 1e-5
GROUPS = ((0, 1, 2), (3, 4, 5))  # resident expert groups

_PROG_CACHE = {}
LAST_RESULTS = None  # BassKernelResults of the most recent run (for test.py)


def _build_program(mm_dtype_name: str):
    import concourse.bass as bass
    import concourse.mybir as mybir
    import concourse.tile as tile
    from concourse import bacc
    from concourse.bass import ds

    f32 = mybir.dt.float32
    mmdt = getattr(mybir.dt, mm_dtype_name)
    AF = mybir.ActivationFunctionType
    ALU = mybir.AluOpType

    mdt = mmdt  # dtype for all matmul-feeding tensors
    nc = bacc.Bacc(
        bass.get_trn_type() or "TRN2",
        target_bir_lowering=False,
        debug=False,
        num_devices=N_CORES,
    )

    xT = nc.declare_dram_parameter("xT", [D, NTOK], mdt, isOutput=False)
    w1 = nc.declare_dram_parameter("w1", [NUM_EXPERTS, D, FC], mdt, isOutput=False)
    w2q = nc.declare_dram_parameter("w2q", [NUM_EXPERTS, FC, D], mdt, isOutput=False)
    b1 = nc.declare_dram_parameter("b1", [NUM_EXPERTS, FC], f32, isOutput=False)
    b2q = nc.declare_dram_parameter("b2q", [NUM_EXPERTS, D], mdt, isOutput=False)
    maskT = nc.declare_dram_parameter("maskT", [NUM_EXPERTS, NTOK], mdt, isOutput=False)
    gbc = nc.declare_dram_parameter("gbc", [128, D], f32, isOutput=False)
    bbc = nc.declare_dram_parameter("bbc", [128, D], f32, isOutput=False)
    out = nc.declare_dram_parameter("out", [TOK_SLICE, D], f32, isOutput=True)

    with tile.TileContext(nc) as tc:
        with (
            tc.tile_pool(name="wpool", bufs=1) as wpool,
            tc.tile_pool(name="xpool", bufs=2) as xpool,
            tc.tile_pool(name="hpool", bufs=2) as hpool,
            tc.tile_pool(name="mpool", bufs=2) as mpool,
            tc.tile_pool(name="opool", bufs=2) as opool,
            tc.tile_pool(name="cpool", bufs=1) as cpool,
            tc.tile_pool(name="lnpool", bufs=1) as lnpool,
            tc.tile_pool(name="ph_pool", bufs=4, space="PSUM") as ph_pool,
            tc.tile_pool(name="po_pool", bufs=4, space="PSUM") as po_pool,
            tc.tile_pool(name="dram", bufs=1, space="DRAM") as dram,
        ):
            # ---- constants / small tensors resident for the whole kernel ----
            b2q_sb = cpool.tile([NUM_EXPERTS, D], mdt, tag="b2q_sb")
            nc.sync.dma_start(b2q_sb[:], b2q[:])
            eps_sb = cpool.tile([128, 1], f32, tag="eps_sb")
            nc.vector.memset(eps_sb[:], LN_EPS)
            gbc_sb = cpool.tile([128, D], f32, tag="gbc_sb")
            nc.sync.dma_start(gbc_sb[:], gbc[:])
            bbc_sb = cpool.tile([128, D], f32, tag="bbc_sb")
            nc.sync.dma_start(bbc_sb[:], bbc[:])

            acc = dram.tile([NTOK, D], f32, tag="acc")
            rs_out = dram.tile([TOK_SLICE, D], f32, tag="rs_out")

            for gi, grp in enumerate(GROUPS):
                # ---- load this group's weight slabs ----
                w1_sb, w2_sb, b1_sb = [], [], []
                for i, e in enumerate(grp):
                    t1 = wpool.tile([128, D // 128, FC], mdt, tag=f"w1_{i}")
                    nc.sync.dma_start(
                        t1[:], w1[e].rearrange("(dt p) f -> p dt f", p=128)
                    )
                    w1_sb.append(t1)
                    t2 = wpool.tile([128, FC // 128, D], mdt, tag=f"w2_{i}")
                    nc.sync.dma_start(
                        t2[:], w2q[e].rearrange("(ft p) d -> p ft d", p=128)
                    )
                    w2_sb.append(t2)
                    tb = wpool.tile([128, FC // 128], f32, tag=f"b1_{i}")
                    nc.sync.dma_start(tb[:], b1[e].rearrange("(ft p) -> p ft", p=128))
                    b1_sb.append(tb)

                for tci in range(NTC):
                    tok0 = tci * TOKC
                    xt = xpool.tile([128, D // 128, TOKC], mdt, tag="xt")
                    nc.sync.dma_start(
                        xt[:],
                        xT[:, ds(tok0, TOKC)].rearrange("(dt p) t -> p dt t", p=128),
                    )

                    # broadcast masks for the variable experts of this group
                    mbs = {}
                    for i, e in enumerate(grp):
                        if e < FIXED_EXPERTS:
                            continue
                        mb = mpool.tile([128, TOKC], mdt, tag=f"mb_{i}")
                        nc.sync.dma_start(
                            mb[:],
                            maskT[e, ds(tok0, TOKC)].partition_broadcast(128),
                        )
                        mbs[i] = mb
                    if gi == 0:
                        mk = mpool.tile([NUM_EXPERTS, TOKC], mdt, tag="mk")
                        nc.sync.dma_start(mk[:], maskT[:, ds(tok0, TOKC)])

                    # ---- stage A: hT[e] = mask * gelu(W1[e].T @ x + b1) ----
                    hts = {}
                    for i, e in enumerate(grp):
                        for ft in range(FC // 128):
                            ph = ph_pool.tile([128, TOKC], f32, tag="ph")
                            for dt in range(D // 128):
                                nc.tensor.matmul(
                                    ph[:],
                                    (w1_sb[i][:, dt, ds(ft * 128, 128)]),
                                    (xt[:, dt, :]),
                                    start=(dt == 0),
                                    stop=(dt == D // 128 - 1),
                                )
                            ht = hpool.tile([128, TOKC], mdt, tag=f"ht_{i}_{ft}")
                            nc.scalar.activation(
                                ht[:], ph[:], AF.Gelu, bias=b1_sb[i][:, ft : ft + 1]
                            )
                            if i in mbs:
                                nc.vector.tensor_tensor(
                                    ht[:], ht[:], mbs[i][:], op=ALU.mult
                                )
                            hts[(i, ft)] = ht

                    # ---- stage B: out partial += sum_e hT[e].T @ W2[e] ----
                    for tt in range(TOKC // 128):
                        trow = tok0 + tt * 128
                        for nd in range(D // 512):
                            po = po_pool.tile([128, 512], f32, tag="po")
                            nmm = len(grp) * (FC // 128)
                            k = 0
                            for i in range(len(grp)):
                                for ft in range(FC // 128):
                                    nc.tensor.matmul(
                                        po[:],
                                        (hts[(i, ft)][:, ds(tt * 128, 128)]),
                                        (w2_sb[i][:, ft, ds(nd * 512, 512)]),
                                        start=(k == 0),
                                        stop=(k == nmm - 1 and gi != 0),
                                    )
                                    k += 1
                            if gi == 0:
                                # bias term: mask.T @ (b2/4), counted once
                                nc.tensor.matmul(
                                    po[:],
                                    (mk[:, ds(tt * 128, 128)]),
                                    (b2q_sb[:, ds(nd * 512, 512)]),
                                    start=False,
                                    stop=True,
                                )
                                ob = opool.tile([128, 512], f32, tag="ob")
                                nc.scalar.copy(ob[:], po[:])
                            else:
                                prev = opool.tile([128, 512], f32, tag="prev")
                                nc.sync.dma_start(
                                    prev[:], acc[ds(trow, 128), ds(nd * 512, 512)]
                                )
                                ob = opool.tile([128, 512], f32, tag="ob")
                                nc.vector.tensor_tensor(
                                    ob[:], po[:], prev[:], op=ALU.add
                                )
                            nc.sync.dma_start(
                                acc[ds(trow, 128), ds(nd * 512, 512)], ob[:]
                            )

            # ---- ReduceScatter partials over all 8 cores ----
            nc.gpsimd.collective_compute(
                "ReduceScatter",
                mybir.AluOpType.add,
                replica_groups=[list(range(N_CORES))],
                ins=[acc.opt()],
                outs=[rs_out.opt()],
            )

            # ---- LayerNorm on this core's 512-token slice ----
            for t in range(TOK_SLICE // 128):
                z = lnpool.tile([128, D], f32, tag="z")
                nc.sync.dma_start(z[:], rs_out[ds(t * 128, 128), :])
                ssum = lnpool.tile([128, 1], f32, tag="ssum")
                nc.vector.reduce_sum(out=ssum[:], in_=z[:], axis=mybir.AxisListType.X)
                negmu = lnpool.tile([128, 1], f32, tag="negmu")
                nc.scalar.mul(negmu[:], ssum[:], -1.0 / D)
                zc = lnpool.tile([128, D], f32, tag="zc")
                nc.scalar.activation(zc[:], z[:], AF.Identity, bias=negmu[:])
                zn = lnpool.tile([128, D], f32, tag="zn")
                ssq = lnpool.tile([128, 1], f32, tag="ssq")
                nc.scalar.activation(zn[:], zc[:], AF.Square, accum_out=ssq[:])
                std = lnpool.tile([128, 1], f32, tag="std")
                nc.scalar.activation(
                    std[:], ssq[:], AF.Sqrt, bias=eps_sb[:], scale=1.0 / D
                )
                rstd = lnpool.tile([128, 1], f32, tag="rstd")
                nc.vector.reciprocal(rstd[:], std[:])
                nc.scalar.activation(zn[:], zc[:], AF.Identity, scale=rstd[:])
                nc.vector.tensor_mul(zc[:], zn[:], gbc_sb[:])
                nc.vector.tensor_add(zn[:], zc[:], bbc_sb[:])
                nc.sync.dma_start(out[ds(t * 128, 128), :], zn[:])

    nc.finalize()
    return nc


def _route(x: np.ndarray, router_W: np.ndarray):
    """Replicate the reference's router exactly: softmax top-2 over 4 variable
    experts.  Returns (router_logits (B,S,4) f32, maskT (6, NTOK) f32)."""
    try:
        import jax
        import jax.numpy as jnp

        cpu = jax.devices("cpu")[0]
        with jax.default_device(cpu):
            logits = jnp.einsum(
                "bsd,de->bse", jnp.asarray(x), jnp.asarray(router_W)
            )
            probs = jax.nn.softmax(logits, axis=-1)
            _, idx = jax.lax.top_k(probs, TOP_K)
            logits_np = np.asarray(logits, dtype=np.float32)
            idx_np = np.asarray(idx)
    except Exception:
        logits_np = np.einsum(
            "bsd,de->bse", x.astype(np.float32), router_W.astype(np.float32)
        ).astype(np.float32)
        order = np.argsort(-logits_np.reshape(-1, VAR_EXPERTS), axis=-1, kind="stable")
        idx_np = order[:, :TOP_K]
    adj = idx_np.reshape(-1, TOP_K) + FIXED_EXPERTS
    maskT_np = np.zeros((NUM_EXPERTS, NTOK), dtype=np.float32)
    maskT_np[:FIXED_EXPERTS, :] = 1.0
    tok = np.arange(NTOK)
    for k in range(TOP_K):
        maskT_np[adj[:, k], tok] = 1.0
    return logits_np, maskT_np


def prepare(x, router_W, W1, b1, W2, b2, ln_gamma, ln_beta):
    """Build (nc, in_maps, router_logits) for the current MOE_MM_DTYPE."""
    mm_dtype = os.environ.get("MOE_MM_DTYPE", "float32r")
    if mm_dtype not in _PROG_CACHE:
        _PROG_CACHE[mm_dtype] = _build_program(mm_dtype)
    nc = _PROG_CACHE[mm_dtype]

    x = np.asarray(x, dtype=np.float32)
    router_W = np.asarray(router_W, dtype=np.float32)
    W1 = np.asarray(W1, dtype=np.float32)
    b1 = np.asarray(b1, dtype=np.float32)
    W2 = np.asarray(W2, dtype=np.float32)
    b2 = np.asarray(b2, dtype=np.float32)
    ln_gamma = np.asarray(ln_gamma, dtype=np.float32)
    ln_beta = np.asarray(ln_beta, dtype=np.float32)

    router_logits, maskT_np = _route(x, router_W)

    xflat = x.reshape(NTOK, D)
    xT_np = np.ascontiguousarray(xflat.T)
    w2q_full = W2 * 0.25  # exact: folds the /(FIXED+TOP_K) combine divide
    b2q_np = np.ascontiguousarray(b2 * 0.25)
    gbc_np = np.ascontiguousarray(np.broadcast_to(ln_gamma, (128, D)))
    bbc_np = np.ascontiguousarray(np.broadcast_to(ln_beta, (128, D)))

    in_maps = []
    for c in range(N_CORES):
        sl = slice(c * FC, (c + 1) * FC)
        in_maps.append(
            {
                "xT": xT_np,
                "w1": np.ascontiguousarray(W1[:, :, sl]),
                "w2q": np.ascontiguousarray(w2q_full[:, sl, :]),
                "b1": np.ascontiguousarray(b1[:, sl]),
                "b2q": b2q_np,
                "maskT": maskT_np,
                "gbc": gbc_np,
                "bbc": bbc_np,
            }
        )

    return nc, in_maps, router_logits


def kernel(x, router_W, W1, b1, W2, b2, ln_gamma, ln_beta):
    global LAST_RESULTS
    from concourse.bass_utils import run_bass_kernel_spmd

    nc, in_maps, router_logits = prepare(
        x, router_W, W1, b1, W2, b2, ln_gamma, ln_beta
    )
    res = run_bass_kernel_spmd(nc, in_maps, list(range(N_CORES)))
    LAST_RESULTS = res
    out = np.concatenate([res.results[c]["out"] for c in range(N_CORES)], axis=0)
    return out.reshape(B, S, D), router_logits
